# revision 1
# baseline (speedup 1.0000x reference)
"""GQA (32 q heads / 8 kv heads, RoPE, causal) Trainium2 Bass kernel.

Sharding: tensor-parallel over kv heads — core c owns kv head c and q heads
4c..4c+3 for both batches. Each core computes a partial o-projection
(its 256 attn channels x Wo columns) and the host sums the 8 partials.

Device-side structure (per core, per batch):
  * Fused QKV projection: one accumulation chain per 128-token tile produces
    [t, 384] = [4 q heads | k head | v head] with d contracted on partitions
    (host passes x pre-transposed).  float32r matmuls (1 cycle/row).
  * RoPE applied in token-partition layout with stride-2 free-dim APs
    (interleaved even/odd pairs), 6 DVE ops per tile covering all 5 heads.
  * Q/K transposed per-head via TensorE into [dh, t] (f32r), V kept natural
    [t, dh] with a ones column appended.
  * Scores computed transposed [keys, queries]; exp on ACT (no max needed:
    |scores| small by construction); causal diagonal masked by DVE multiply.
  * attn.V matmul gives attnT [dh, i] plus the softmax denominator for free
    (ones row of V); normalization via reciprocal + PE broadcast + DVE mul
    writes attnT directly into the o-projection's stationary layout [c, t].
"""

import numpy as np
from contextlib import ExitStack

import concourse.bass as bass
from concourse import bacc
import concourse.mybir as mybir
import concourse.tile as tile
from concourse.bass_utils import run_bass_kernel_spmd

B, S, D = 2, 2048, 2048
DH = 64            # head dim
G = 4              # q heads per core (= per kv head)
NCORES = 8
TT = 512           # attention i-tile
NTT = S // TT      # 4
KC = D // 128      # 16 contraction chunks
NJC = S // 128     # 16 token/key chunks of 128
F32 = mybir.dt.float32
F32R = mybir.dt.float32r
ROPE_BASE = 10000.0

_cached = {}


def build_nc():
    nc = bacc.Bacc("TRN2", target_bir_lowering=False, debug=False)
    xt = nc.declare_dram_parameter("xt", [B, D, S], F32, isOutput=False)
    wall = nc.declare_dram_parameter("wall", [D, 384], F32, isOutput=False)
    wot = nc.declare_dram_parameter("wot", [256, D], F32, isOutput=False)
    cosr = nc.declare_dram_parameter("cosr", [S, 160], F32, isOutput=False)
    sinr = nc.declare_dram_parameter("sinr", [S, 160], F32, isOutput=False)
    cmask = nc.declare_dram_parameter("cmask", [4, 128, TT], F32, isOutput=False)
    ident = nc.declare_dram_parameter("ident", [128, 128], F32, isOutput=False)
    o = nc.declare_dram_parameter("o", [B, S, D], F32, isOutput=True)

    EXP = mybir.ActivationFunctionType.Exp

    with tile.TileContext(nc) as tc, ExitStack() as ctx:
        wpool = ctx.enter_context(tc.tile_pool(name="weights", bufs=1))
        per_b = ctx.enter_context(tc.tile_pool(name="per_b", bufs=1))
        xpool = ctx.enter_context(tc.tile_pool(name="xstream", bufs=12))
        qkvpool = ctx.enter_context(tc.tile_pool(name="qkv", bufs=3))
        epool = ctx.enter_context(tc.tile_pool(name="exp", bufs=6))
        rpool = ctx.enter_context(tc.tile_pool(name="rope", bufs=2))
        opool = ctx.enter_context(tc.tile_pool(name="out", bufs=4))
        spool = ctx.enter_context(tc.tile_pool(name="small", bufs=4))
        pp_proj = ctx.enter_context(tc.tile_pool(name="pproj", bufs=1, space="PSUM"))
        pp_att = ctx.enter_context(tc.tile_pool(name="patt", bufs=2, space="PSUM"))
        pp_av = ctx.enter_context(tc.tile_pool(name="pav", bufs=1, space="PSUM"))
        pp_misc = ctx.enter_context(tc.tile_pool(name="pmisc", bufs=1, space="PSUM"))

        # ---- persistent weights/tables ----
        wall_sb = wpool.tile([128, KC, 384], F32R, tag="wall")
        wot_sb = wpool.tile([128, 2, D], F32R, tag="wot")
        cos_sb = wpool.tile([128, NJC, 160], F32, tag="cos")
        sin_sb = wpool.tile([128, NJC, 160], F32, tag="sin")
        mask_sb = wpool.tile([128, 4, TT], F32R, tag="mask")
        ident_sb = wpool.tile([128, 128], F32, tag="ident")
        ones_sb = wpool.tile([1, 64], F32R, tag="ones")
        for k in range(KC):
            nc.sync.dma_start(wall_sb[:, k, :],
                              wall[k * 128:(k + 1) * 128, :].bitcast(F32R))
        for cc in range(2):
            nc.sync.dma_start(wot_sb[:, cc, :],
                              wot[cc * 128:(cc + 1) * 128, :].bitcast(F32R))
        for j in range(NJC):
            nc.sync.dma_start(cos_sb[:, j, :], cosr[j * 128:(j + 1) * 128, :])
            nc.sync.dma_start(sin_sb[:, j, :], sinr[j * 128:(j + 1) * 128, :])
        for m in range(4):
            nc.sync.dma_start(mask_sb[:, m, :], cmask[m].bitcast(F32R))
        nc.sync.dma_start(ident_sb[:], ident[:, :])
        nc.vector.memset(ones_sb[:].bitcast(F32), 1.0)

        for b in range(B):
            qt = per_b.tile([64, G, S], F32R, tag="qt")
            kt = per_b.tile([64, S], F32R, tag="kt")
            vsb = per_b.tile([128, NJC, DH + 1], F32R, tag="vsb")
            at = per_b.tile([128, 2, S], F32R, tag="at")
            nc.vector.memset(vsb[:].bitcast(F32), 1.0)

            # ---------- fused QKV projection + rope + transposes ----------
            # Transposes for tile tt are emitted after tile tt+1's matmuls so
            # the PE never waits on the ACT-evict -> DVE-rope chain.
            def emit_tail(tt, qkv):
                tsl = slice(tt * 128, (tt + 1) * 128)
                for h in range(5):
                    ptr = pp_misc.tile([64, 128], F32, tag="misc")
                    nc.tensor.transpose(ptr[:], qkv[:, h * 64:(h + 1) * 64],
                                        ident_sb[:, :])
                    if h < G:
                        nc.vector.tensor_copy(qt[:, h, tsl], ptr[:])
                    else:
                        nc.vector.tensor_copy(kt[:, tsl], ptr[:])
                nc.vector.tensor_copy(vsb[:, tt, 0:DH], qkv[:, 320:384])

            prev = None
            for tg in range(4):             # groups of 512 tokens, 4 psum accs
                pq = [pp_proj.tile([128, 384], F32, tag=f"pq{s}",
                                   name=f"pq{s}_{b}_{tg}")
                      for s in range(4)]
                for k in range(KC):
                    xbig = xpool.tile([128, 512], F32R, tag="xt")
                    nc.sync.dma_start(
                        xbig[:],
                        xt[b, k * 128:(k + 1) * 128,
                           tg * 512:(tg + 1) * 512].bitcast(F32R))
                    for s in range(4):
                        nc.tensor.matmul(pq[s][:],
                                         xbig[:, s * 128:(s + 1) * 128],
                                         wall_sb[:, k, :],
                                         start=(k == 0), stop=(k == KC - 1))
                for s in range(4):
                    tt = tg * 4 + s
                    qkv = qkvpool.tile([128, 384], F32, tag="qkv")
                    nc.scalar.copy(qkv[:], pq[s][:])
                    # rope on q+k (cols 0:320), interleaved pairs in free dim
                    pear = qkv[:, 0:320].rearrange("p (h i two) -> p h i two",
                                                   two=2, i=32)
                    ev, od = pear[:, :, :, 0], pear[:, :, :, 1]
                    cs = cos_sb[:, tt, :].rearrange("p (h i) -> p h i", i=32)
                    sn = sin_sb[:, tt, :].rearrange("p (h i) -> p h i", i=32)
                    ec = rpool.tile([128, 5, 32], F32, tag="ec")
                    es = rpool.tile([128, 5, 32], F32, tag="es")
                    oc = rpool.tile([128, 5, 32], F32, tag="oc")
                    os_ = rpool.tile([128, 5, 32], F32, tag="os")
                    nc.vector.tensor_mul(ec[:], ev, cs)
                    nc.vector.tensor_mul(es[:], ev, sn)
                    nc.vector.tensor_mul(oc[:], od, cs)
                    nc.vector.tensor_mul(os_[:], od, sn)
                    nc.vector.tensor_sub(ev, ec[:], os_[:])
                    nc.vector.tensor_add(od, es[:], oc[:])
                    if prev is not None:
                        emit_tail(*prev)
                    prev = (tt, qkv)
            emit_tail(*prev)

            # ---------- attention ----------
            for g in range(G):
                cc, r0 = g // 2, (g % 2) * 64
                for it in range(NTT):
                    isl = slice(it * TT, (it + 1) * TT)
                    pav = pp_av.tile([65, TT], F32, tag="av")
                    njc = 4 * it + 4
                    pending = []  # attn.V pipelined two steps behind scores
                    for jc in range(njc):
                        psc = pp_att.tile([128, TT], F32, tag="sc")
                        nc.tensor.matmul(
                            psc[:], kt[:, jc * 128:(jc + 1) * 128],
                            qt[:, g, isl], start=True, stop=True)
                        esb = epool.tile([128, TT], F32R, tag="exp")
                        nc.scalar.activation(esb[:], psc[:], EXP, scale=0.125)
                        if jc >= 4 * it:  # diagonal block: causal mask
                            nc.vector.tensor_mul(esb[:], esb[:],
                                                 mask_sb[:, jc - 4 * it, :])
                        pending.append(((pav[:], vsb[:, jc, :], esb[:]),
                                        dict(start=(jc == 0),
                                             stop=(jc == njc - 1))))
                        if len(pending) > 2:
                            a = pending.pop(0)
                            nc.tensor.matmul(*a[0], **a[1])
                    for a in pending:
                        nc.tensor.matmul(*a[0], **a[1])
                    # normalize via ones-row sum: recip -> PE broadcast -> mul
                    rcp = spool.tile([1, TT], F32, tag="rcp")
                    nc.vector.reciprocal(rcp[:], pav[64:65, :])
                    avs = spool.tile([64, TT], F32, tag="avs")
                    nc.scalar.copy(avs[:], pav[0:64, :])
                    rcpr = spool.tile([1, TT], F32R, tag="rcpr")
                    nc.vector.tensor_copy(rcpr[:], rcp[:])
                    pbc = pp_misc.tile([64, TT], F32, tag="misc")
                    nc.tensor.matmul(pbc[:], ones_sb[:], rcpr[:],
                                     start=True, stop=True)
                    nc.vector.tensor_mul(at[r0:r0 + 64, cc, isl],
                                         avs[:], pbc[:])

            # ---------- o projection (partial over this core's channels) ----
            for tt in range(NJC):
                tsl = slice(tt * 128, (tt + 1) * 128)
                for nt in range(D // TT):
                    nsl = slice(nt * TT, (nt + 1) * TT)
                    po = pp_proj.tile([128, TT], F32, tag=f"pq{nt}",
                                      name=f"po{b}_{tt}_{nt}")
                    nc.tensor.matmul(po[:], at[:, 0, tsl], wot_sb[:, 0, nsl],
                                     start=True, stop=False)
                    nc.tensor.matmul(po[:], at[:, 1, tsl], wot_sb[:, 1, nsl],
                                     start=False, stop=True)
                    osb = opool.tile([128, TT], F32, tag="osb")
                    nc.vector.tensor_copy(osb[:], po[:])
                    nc.sync.dma_start(o[b, tsl, nsl], osb[:])
    nc.compile()
    return nc


def host_inputs(x, Wq, Wk, Wv, Wo):
    """Per-core input maps. Q/K weight rows permuted so each head is
    [interleaved] kept natural; rope works on interleaved pairs in the
    free dim, so NO permutation is needed here."""
    xtp = np.ascontiguousarray(np.transpose(np.asarray(x, np.float32), (0, 2, 1)))
    inv = ROPE_BASE ** (-np.arange(0, DH, 2, dtype=np.float64) / DH)
    th = np.arange(S, dtype=np.float64)[:, None] * inv[None, :]  # (S, 32)
    cosr = np.tile(np.cos(th), (1, 5)).astype(np.float32)  # (S, 160)
    sinr = np.tile(np.sin(th), (1, 5)).astype(np.float32)
    p = np.arange(128)[:, None]
    f = np.arange(TT)[None, :]
    cmask = np.stack([(p + m * 128 <= f).astype(np.float32) for m in range(4)])
    ident = np.eye(128, dtype=np.float32)
    in_maps = []
    for c in range(NCORES):
        wall = np.concatenate([Wq[256 * c:256 * (c + 1)],
                               Wk[DH * c:DH * (c + 1)],
                               Wv[DH * c:DH * (c + 1)]], axis=0)
        wall = np.ascontiguousarray(wall.T.astype(np.float32))       # (D, 384)
        wot = np.ascontiguousarray(Wo[:, 256 * c:256 * (c + 1)].T
                                   .astype(np.float32))              # (256, D)
        in_maps.append(dict(xt=xtp, wall=wall, wot=wot, cosr=cosr,
                            sinr=sinr, cmask=cmask, ident=ident))
    return in_maps


def kernel(**inputs):
    x = np.asarray(inputs["x"], dtype=np.float32)
    Wq = np.asarray(inputs["Wq"], dtype=np.float32)
    Wk = np.asarray(inputs["Wk"], dtype=np.float32)
    Wv = np.asarray(inputs["Wv"], dtype=np.float32)
    Wo = np.asarray(inputs["Wo"], dtype=np.float32)
    in_maps = host_inputs(x, Wq, Wk, Wv, Wo)
    if "nc" not in _cached:
        _cached["nc"] = build_nc()
    res = run_bass_kernel_spmd(_cached["nc"], in_maps, list(range(NCORES)))
    out = np.zeros((B, S, D), np.float64)
    for r in res.results:
        out += r["o"]
    return out.astype(np.float32)



# revision 9
# speedup vs baseline: 1.5030x; 1.5030x over previous
"""GQA (32 q heads / 8 kv heads, RoPE, causal) Trainium2 Bass kernel.

Sharding: tensor-parallel over kv heads — core c owns kv head c and q heads
4c..4c+3 for both batches. Each core computes a partial o-projection
(its 256 attn channels x all Wo columns) in bf16 and the host sums the 8
partials in f32.

Device-side structure (per core), 4-stage software pipeline over 128-token
chunks p (32 chunks across both batches) so the PE never waits on the
ACT/DVE rope/copy chains:
  iter i:  proj-mm(p=i) -> attn-transposes(r=i-3) -> QK-transposes(q=i-1)
           -> rope-muls(p) -> o-proj(r) -> scores/exp/AV(q) -> rope-comb(p)
  * QKV projection: bf16 x^T chunks (stationary) x bf16 fused W (moving 384)
    into f32 PSUM. RoPE reads the PSUM directly (DVE, f32, with a
    host-deinterleaved pair layout = contiguous even/odd halves) and writes
    a bf16 tile for the transposes — no separate PSUM evacuation.
  * Q/K transposed per head via PE (bf16, 1 c/row) into [dh, token] layout
    (k stored as head 4 of the same tensor); 4 q heads merged along the
    moving dim for scores.
  * Scores at 128x(4x128) causal granularity (no above-diagonal waste);
    exp on ACT (scale=1/8, no max subtraction needed); diagonal masked by
    one bf16 GPSIMD multiply.
  * AV in [query, dh] layout: out [128q, 65] per head with a ones column in
    V giving the softmax denominator; full 128-partition output = 2x fewer
    PE cycles than the [dh, query] layout. Normalization = DVE reciprocal +
    per-partition tensor_scalar multiplies (no PE broadcast).
  * attn transposed back to [chan, token] (2 heads per transpose) for the
    o-proj; o written as bf16 via Pool-issued (SWDGE) DMAs to avoid the
    serialized HWDGE path.
"""

import numpy as np
from contextlib import ExitStack

import concourse.bass as bass
from concourse import bacc
import concourse.mybir as mybir
import concourse.tile as tile
from concourse.bass_utils import run_bass_kernel_spmd

B, S, D = 2, 2048, 2048
DH = 64            # head dim
G = 4              # q heads per core (= per kv head)
NCORES = 8
NP = S // 128      # 16 token chunks of 128 per batch
KC = D // 128      # 16 contraction chunks
F32 = mybir.dt.float32
F32R = mybir.dt.float32r
BF16 = mybir.dt.bfloat16
ROPE_BASE = 10000.0

_cached = {}


def build_nc():
    nc = bacc.Bacc("TRN2", target_bir_lowering=False, debug=False)
    xt = nc.declare_dram_parameter("xt", [B, 128, KC, S], BF16, isOutput=False)
    wall = nc.declare_dram_parameter("wall", [128, KC, 384], BF16, isOutput=False)
    wot = nc.declare_dram_parameter("wot", [128, 2, D], BF16, isOutput=False)
    cosr = nc.declare_dram_parameter("cosr", [128, NP, 160], F32, isOutput=False)
    sinr = nc.declare_dram_parameter("sinr", [128, NP, 160], F32, isOutput=False)
    maskd = nc.declare_dram_parameter("maskd", [128, 512], BF16, isOutput=False)
    identb_d = nc.declare_dram_parameter("identb", [128, 128], BF16, isOutput=False)
    o = nc.declare_dram_parameter("o", [B, S, D], BF16, isOutput=True)

    EXP = mybir.ActivationFunctionType.Exp

    with tile.TileContext(nc) as tc, ExitStack() as ctx:
        wpool = ctx.enter_context(tc.tile_pool(name="weights", bufs=1))
        xpool = ctx.enter_context(tc.tile_pool(name="x", bufs=2))
        qkvpool = ctx.enter_context(tc.tile_pool(name="qkvb", bufs=3))
        rpool = ctx.enter_context(tc.tile_pool(name="rope", bufs=2))
        epool = ctx.enter_context(tc.tile_pool(name="exp", bufs=18))
        bpool = ctx.enter_context(tc.tile_pool(name="perb", bufs=2))
        apool = ctx.enter_context(tc.tile_pool(name="attn", bufs=4))
        spool = ctx.enter_context(tc.tile_pool(name="small", bufs=4))
        opool = ctx.enter_context(tc.tile_pool(name="osb", bufs=2))
        pp_pq = ctx.enter_context(tc.tile_pool(name="ppq", bufs=1, space="PSUM"))
        pp_sc = ctx.enter_context(tc.tile_pool(name="psc", bufs=3, space="PSUM"))
        pp_av = ctx.enter_context(tc.tile_pool(name="pav", bufs=1, space="PSUM"))
        pp_tr = ctx.enter_context(tc.tile_pool(name="ptr", bufs=1, space="PSUM"))
        pp_po = ctx.enter_context(tc.tile_pool(name="ppo", bufs=2, space="PSUM"))

        # ---- persistent weights/tables ----
        # wall first (needed by proj(0)); the rest are emitted after the
        # first x-tile DMA so startup isn't serialized behind weight loads
        # the first iterations don't need yet.
        wall_sb = wpool.tile([128, KC, 384], BF16, tag="wall")
        wot_sb = wpool.tile([128, 2, D], BF16, tag="wot")
        cos_sb = wpool.tile([128, NP, 160], F32, tag="cos")
        sin_sb = wpool.tile([128, NP, 160], F32, tag="sin")
        mask_sb = wpool.tile([128, 512], BF16, tag="mask")
        identb = wpool.tile([128, 128], BF16, tag="identb")
        nc.sync.dma_start(wall_sb[:], wall[:, :, :])

        def emit_tables():
            nc.sync.dma_start(cos_sb[:], cosr[:, :, :])
            nc.sync.dma_start(sin_sb[:], sinr[:, :, :])
            nc.sync.dma_start(identb[:], identb_d[:, :])
            nc.sync.dma_start(mask_sb[:], maskd[:, :])
            nc.sync.dma_start(wot_sb[:], wot[:, :, :])

        # per-chunk state, filled by emit stages
        C = [dict() for _ in range(B * NP)]
        xtiles = {}
        btiles = {}

        def emit_proj(gi):
            b, p = gi // NP, gi % NP
            if p == 0:
                # qt holds the 4 roped q heads AND k (slot 4) in [dh, token]
                qt = bpool.tile([64, NP, 5, 128], BF16, tag="qt", name=f"qt{b}")
                vsb = bpool.tile([128, NP, DH + 1], BF16, tag="vsb", name=f"vsb{b}")
                at = bpool.tile([128, 2, S], BF16, tag="at", name=f"at{b}")
                nc.gpsimd.memset(vsb[:], 1.0)
                btiles[b] = (qt, vsb, at)
            if p % 4 == 0:
                tg = p // 4
                xtg = xpool.tile([128, KC, 512], BF16, tag="x", name=f"x{b}_{tg}")
                nc.sync.dma_start(xtg[:], xt[b, :, :, tg * 512:(tg + 1) * 512])
                xtiles[(b, tg)] = xtg
                if gi == 0:
                    emit_tables()
            xtg = xtiles[(b, p // 4)]
            s = p % 4
            pq = pp_pq.tile([128, 384], F32, tag="pq", name=f"pq{b}_{p}")
            for kc in range(KC):
                nc.tensor.matmul(pq[:], xtg[:, kc, s * 128:(s + 1) * 128],
                                 wall_sb[:, kc, :],
                                 start=(kc == 0), stop=(kc == KC - 1))
            C[gi].update(b=b, p=p, pq=pq, bt=btiles[b])

        def emit_rope_muls(gi):
            pq, p = C[gi]["pq"], C[gi]["p"]
            qt, vsb, at = C[gi]["bt"]
            pear = pq[:, 0:320].rearrange("p (h two i) -> p h two i", two=2, i=32)
            ev, od = pear[:, :, 0, :], pear[:, :, 1, :]
            cs = cos_sb[:, p, :].rearrange("p (h i) -> p h i", i=32)
            sn = sin_sb[:, p, :].rearrange("p (h i) -> p h i", i=32)
            ec = rpool.tile([128, 5, 32], F32, tag="ec")
            es = rpool.tile([128, 5, 32], F32, tag="es")
            oc = rpool.tile([128, 5, 32], F32, tag="oc")
            os_ = rpool.tile([128, 5, 32], F32, tag="os")
            nc.vector.tensor_mul(ec[:], ev, cs)
            nc.vector.tensor_mul(es[:], ev, sn)
            nc.vector.tensor_mul(oc[:], od, cs)
            nc.vector.tensor_mul(os_[:], od, sn)
            nc.vector.tensor_copy(vsb[:, p, 0:DH], pq[:, 320:384])
            C[gi].update(ec=ec, es=es, oc=oc, os=os_)

        def emit_rope_comb(gi):
            c = C[gi]
            qkvb = qkvpool.tile([128, 5, 64], BF16, tag="qkvb", name=f"qkvb{gi}")
            qb = qkvb[:].rearrange("p h (two i) -> p h two i", two=2, i=32)
            nc.vector.tensor_sub(qb[:, :, 0, :], c["ec"][:], c["os"][:])
            nc.vector.tensor_add(qb[:, :, 1, :], c["es"][:], c["oc"][:])
            c["qkvb"] = qkvb

        def emit_tr_copies(gi):
            c = C[gi]
            qkvb, p = c["qkvb"], c["p"]
            qt, vsb, at = c["bt"]
            tr = pp_tr.tile([128, 640], BF16, tag="tr", name=f"tr{gi}")
            for h in range(5):
                nc.tensor.transpose(tr[0:64, h * 128:(h + 1) * 128],
                                    qkvb[:, h, :], identb[:])
            nc.vector.tensor_copy(
                qt[:, p, :, :],
                tr[0:64, 0:640].rearrange("p (h t) -> p h t", h=5))

        def emit_attention(gi):
            c = C[gi]
            p = c["p"]
            qt, vsb, at = c["bt"]
            pav = pp_av.tile([128, G, DH + 1], F32, tag="pav", name=f"pav{gi}")
            esbs = []
            for kc in range(p + 1):
                psc = pp_sc.tile([128, 512], F32, tag="sc")
                nc.tensor.matmul(psc[:], qt[:, kc, 4, :],
                                 qt[:, p, 0:4, :], start=True, stop=True)
                esb = epool.tile([128, 512], BF16, tag="esb",
                                 name=f"esb{gi}_{kc}")
                nc.scalar.activation(esb[:], psc[:], EXP, scale=0.125)
                if kc == p:
                    nc.gpsimd.tensor_mul(esb[:], esb[:], mask_sb[:])
                esbs.append(esb)
            # One OPEN psum accumulation group per bank at a time: run each
            # head's accumulation chain to completion before the next starts.
            for g in range(G):
                for kc in range(p + 1):
                    nc.tensor.matmul(pav[:, g, :],
                                     esbs[kc][:, g * 128:(g + 1) * 128],
                                     vsb[:, kc, :],
                                     start=(kc == 0), stop=(kc == p))
            rcp = spool.tile([128, G], F32, tag="rcp", name=f"rcp{gi}")
            attn = apool.tile([128, G, DH], BF16, tag="attn", name=f"attn{gi}")
            nc.vector.reciprocal(rcp[:], pav[:, :, DH])
            for g in range(G):
                nc.vector.tensor_scalar_mul(attn[:, g, :], pav[:, g, 0:DH],
                                            rcp[:, g:g + 1])
            c["attn"] = attn

        def emit_at_tr(gi):
            c = C[gi]
            p, attn = c["p"], c["attn"]
            qt, vsb, at = c["bt"]
            tsl = slice(p * 128, (p + 1) * 128)
            # two heads per transpose: [128tok, 2*64] -> [128chan, 128tok],
            # landing directly in the o-proj's [chan, token] layout.
            tr2 = pp_tr.tile([128, 640], BF16, tag="tr", name=f"tr2_{gi}")
            for cc in range(2):
                nc.tensor.transpose(tr2[:, cc * 128:(cc + 1) * 128],
                                    attn[:, 2 * cc:2 * cc + 2, :], identb[:])
            nc.vector.tensor_copy(
                at[:, :, tsl],
                tr2[:, 0:256].rearrange("p (c t) -> p c t", c=2))

        def emit_oproj(gi):
            c = C[gi]
            b, p = c["b"], c["p"]
            qt, vsb, at = c["bt"]
            tsl = slice(p * 128, (p + 1) * 128)
            osb = opool.tile([128, D], BF16, tag="osb", name=f"osb{gi}")
            for nt in range(4):
                nsl = slice(nt * 512, (nt + 1) * 512)
                po = pp_po.tile([128, 512], F32, tag="po")
                nc.tensor.matmul(po[:], at[:, 0, tsl], wot_sb[:, 0, nsl],
                                 start=True, stop=False)
                nc.tensor.matmul(po[:], at[:, 1, tsl], wot_sb[:, 1, nsl],
                                 start=False, stop=True)
                if nt == 0:
                    nc.scalar.copy(osb[:, nsl], po[:])
                else:
                    nc.vector.tensor_copy(osb[:, nsl], po[:])
            nc.gpsimd.dma_start(o[b, tsl, :], osb[:])

        NG = B * NP
        for i in range(NG + 3):
            if i < NG:
                emit_proj(i)
            if i - 3 >= 0:
                emit_at_tr(i - 3)
            if 0 <= i - 1 < NG:
                emit_tr_copies(i - 1)
            if i < NG:
                emit_rope_muls(i)
            if i - 3 >= 0:
                emit_oproj(i - 3)
            if 0 <= i - 1 < NG:
                emit_attention(i - 1)
            if i < NG:
                emit_rope_comb(i)
    nc.compile()
    return nc


def _deinter(w):
    """[64, D] head rows, interleaved rope pairs -> [evens(32) | odds(32)]."""
    return np.concatenate([w[0::2], w[1::2]], axis=0)


def host_inputs(x, Wq, Wk, Wv, Wo):
    import ml_dtypes
    bf16 = ml_dtypes.bfloat16
    # [B, S, D] -> [B, 128, KC, S]: partition-major chunks of the d axis
    xtp = np.transpose(np.asarray(x, np.float32), (0, 2, 1))  # [B, D, S]
    xtp = np.ascontiguousarray(
        xtp.reshape(B, KC, 128, S).transpose(0, 2, 1, 3)).astype(bf16)

    inv = ROPE_BASE ** (-np.arange(0, DH, 2, dtype=np.float64) / DH)  # (32,)
    th = np.arange(S, dtype=np.float64)[:, None] * inv[None, :]       # (S, 32)
    cosr = np.tile(np.cos(th), (1, 5)).astype(np.float32)             # (S, 160)
    sinr = np.tile(np.sin(th), (1, 5)).astype(np.float32)
    cosr = np.ascontiguousarray(
        cosr.reshape(NP, 128, 160).transpose(1, 0, 2))                # (128,16,160)
    sinr = np.ascontiguousarray(sinr.reshape(NP, 128, 160).transpose(1, 0, 2))

    k_ = np.arange(128)[:, None]
    q_ = np.arange(128)[None, :]
    tril = (q_ >= k_).astype(np.float32)                               # (128,128)
    maskd = np.ascontiguousarray(np.tile(tril, (1, 4))).astype(bf16)   # (128,512)
    identb = np.eye(128, dtype=np.float32).astype(bf16)

    in_maps = []
    for c in range(NCORES):
        rows = [_deinter(Wq[(4 * c + h) * DH:(4 * c + h + 1) * DH])
                for h in range(G)]
        rows.append(_deinter(Wk[c * DH:(c + 1) * DH]))
        rows.append(Wv[c * DH:(c + 1) * DH])
        wallc = np.concatenate(rows, axis=0).astype(np.float32)        # (384, D)
        wallc = np.ascontiguousarray(
            wallc.T.reshape(KC, 128, 384).transpose(1, 0, 2)).astype(bf16)
        # at[j, cc] holds head 2*cc + j//64, dh j%64 -> wot rows match
        wotc = np.empty((128, 2, D), np.float32)
        for cc in range(2):
            for half in range(2):
                head = 4 * c + 2 * cc + half
                wotc[half * 64:(half + 1) * 64, cc, :] = \
                    Wo[:, head * DH:(head + 1) * DH].T
        in_maps.append(dict(xt=xtp, wall=wallc, wot=wotc.astype(bf16),
                            cosr=cosr, sinr=sinr, maskd=maskd,
                            identb=identb))
    return in_maps


def kernel(**inputs):
    x = np.asarray(inputs["x"], dtype=np.float32)
    Wq = np.asarray(inputs["Wq"], dtype=np.float32)
    Wk = np.asarray(inputs["Wk"], dtype=np.float32)
    Wv = np.asarray(inputs["Wv"], dtype=np.float32)
    Wo = np.asarray(inputs["Wo"], dtype=np.float32)
    in_maps = host_inputs(x, Wq, Wk, Wv, Wo)
    if "nc" not in _cached:
        _cached["nc"] = build_nc()
    res = run_bass_kernel_spmd(_cached["nc"], in_maps, list(range(NCORES)))
    out = np.zeros((B, S, D), np.float32)
    for r in res.results:
        out += np.asarray(r["o"]).astype(np.float32)
    return out


# revision 42
# speedup vs baseline: 1.7648x; 1.1742x over previous
"""GQA (32 q heads / 8 kv heads, RoPE, causal) Trainium2 Bass kernel.

Sharding: tensor-parallel over kv heads — core c owns kv head c and q heads
4c..4c+3 for both batches. Each core computes a partial o-projection
(its 256 attn channels x all Wo columns) in bf16 and the host sums the 8
partials in f32.

Device-side structure (per core), 4-stage software pipeline over 128-token
chunks p (32 chunks across both batches) so the PE never waits on the
ACT/DVE rope/copy chains:
  iter i:  proj-mm(p=i) -> attn-transposes(i-3) -> QK-transposes(q=i-1)
           -> rope(p) -> scores/exp/AV+norm(q) -> o-proj(i-4)
  * QKV projection: bf16 x^T chunks (stationary) x bf16 fused W (moving 384)
    into f32 PSUM. RoPE reads the PSUM directly (DVE, f32, with a
    host-deinterleaved pair layout = contiguous even/odd halves) and writes
    a bf16 tile for the transposes — no separate PSUM evacuation.
  * Q/K transposed per head via PE (bf16, 1 c/row) into [dh, token] layout
    (k stored as head 4 of the same tensor); 4 q heads merged along the
    moving dim for scores.
  * Scores at 128x(4x128) causal granularity (no above-diagonal waste);
    key-chunk-paired exp on ACT (scale=1/8, no max subtraction needed, one
    [128,1024] op per two chunks amortizes the ACT access latency); the
    diagonal tile masked by one bf16 DVE multiply.
  * AV in [query, dh] layout: out [128q, 65] per head with a ones column in
    V giving the softmax denominator; full 128-partition output = 2x fewer
    PE cycles than the [dh, query] layout. Each head's PSUM accumulation
    chain runs to completion before the next starts (a PSUM bank supports
    only one open accumulation group). PSUM banks (pq/scores/pav/po) are
    released by single bulk copies so the next producer never waits on the
    consumer chains; normalization = DVE reciprocal + per-partition
    tensor_scalar multiplies off the SBUF copy (no PE broadcast).
  * attn transposed back to [chan, token] (2 heads per transpose) for the
    o-proj; o written as bf16 via Pool-issued (SWDGE) DMAs to avoid the
    serialized HWDGE path.
"""

import numpy as np
from contextlib import ExitStack

import concourse.bass as bass
from concourse import bacc
import concourse.mybir as mybir
import concourse.tile as tile
from concourse.bass_utils import run_bass_kernel_spmd

B, S, D = 2, 2048, 2048
DH = 64            # head dim
G = 4              # q heads per core (= per kv head)
NCORES = 8
NP = S // 128      # 16 token chunks of 128 per batch
KC = D // 128      # 16 contraction chunks
F32 = mybir.dt.float32
F32R = mybir.dt.float32r
BF16 = mybir.dt.bfloat16
ROPE_BASE = 10000.0

_cached = {}


def build_nc():
    nc = bacc.Bacc("TRN2", target_bir_lowering=False, debug=False)
    xt = nc.declare_dram_parameter("xt", [B, 128, KC, S], BF16, isOutput=False)
    wall = nc.declare_dram_parameter("wall", [128, KC, 384], BF16, isOutput=False)
    wot = nc.declare_dram_parameter("wot", [128, 2, D], BF16, isOutput=False)
    cosr = nc.declare_dram_parameter("cosr", [128, NP, 160], BF16, isOutput=False)
    sinr = nc.declare_dram_parameter("sinr", [128, NP, 160], BF16, isOutput=False)
    utri_d = nc.declare_dram_parameter("utri", [128, 512], BF16, isOutput=False)
    identb_d = nc.declare_dram_parameter("identb", [128, 128], BF16, isOutput=False)
    o = nc.declare_dram_parameter("o", [B, S, D], BF16, isOutput=True)

    EXP = mybir.ActivationFunctionType.Exp

    with tile.TileContext(nc) as tc, ExitStack() as ctx:
        wpool = ctx.enter_context(tc.tile_pool(name="weights", bufs=1))
        xpool = ctx.enter_context(tc.tile_pool(name="x", bufs=2))
        qkvpool = ctx.enter_context(tc.tile_pool(name="qkvb", bufs=3))
        rpool = ctx.enter_context(tc.tile_pool(name="rope", bufs=2))
        epool = ctx.enter_context(tc.tile_pool(name="exp", bufs=12))
        bpool = ctx.enter_context(tc.tile_pool(name="perb", bufs=2))
        apool = ctx.enter_context(tc.tile_pool(name="attn", bufs=4))
        spool = ctx.enter_context(tc.tile_pool(name="small", bufs=4))
        opool = ctx.enter_context(tc.tile_pool(name="osb", bufs=2))
        pp_pq = ctx.enter_context(tc.tile_pool(name="ppq", bufs=1, space="PSUM"))
        pp_sc = ctx.enter_context(tc.tile_pool(name="psc", bufs=2, space="PSUM"))
        pp_av = ctx.enter_context(tc.tile_pool(name="pav", bufs=1, space="PSUM"))
        pp_po = ctx.enter_context(tc.tile_pool(name="ppo", bufs=2, space="PSUM"))

        # ---- persistent weights/tables ----
        # wall first (needed by proj(0)); the rest are emitted after the
        # first x-tile DMA so startup isn't serialized behind weight loads
        # the first iterations don't need yet.
        wall_sb = wpool.tile([128, KC, 384], BF16, tag="wall")
        wot_sb = wpool.tile([128, 2, D], BF16, tag="wot")
        cos_sb = wpool.tile([128, NP, 160], BF16, tag="cos")
        sin_sb = wpool.tile([128, NP, 160], BF16, tag="sin")
        mask_sb = wpool.tile([128, 512], BF16, tag="mask")
        identb = wpool.tile([128, 128], BF16, tag="identb")
        nc.sync.dma_start(wall_sb[:, 0:4, :], wall[:, 0:4, :])

        def emit_tables():
            nc.sync.dma_start(identb[:], identb_d[:, :])
            nc.sync.dma_start(mask_sb[:], utri_d[:, :])
            nc.sync.dma_start(wot_sb[:], wot[:, :, :])

        # per-chunk state, filled by emit stages
        C = [dict() for _ in range(B * NP)]
        xtiles = {}
        btiles = {}

        def emit_proj(gi):
            b, p = gi // NP, gi % NP
            if p == 0:
                # qt holds the 4 roped q heads AND k (slot 4) in [dh, token]
                qt = bpool.tile([64, NP, 5, 128], BF16, tag="qt", name=f"qt{b}")
                vsb = bpool.tile([128, NP, DH + 1], BF16, tag="vsb", name=f"vsb{b}")
                at = bpool.tile([128, 2, S], BF16, tag="at", name=f"at{b}")
                nc.gpsimd.memset(vsb[:], 1.0)
                btiles[b] = (qt, vsb, at)
            def load_x(gj):
                bj, tg = gj // NP, (gj % NP) // 4
                xtg = xpool.tile([128, KC, 512], BF16, tag="x",
                                 name=f"x{bj}_{tg}")
                base = tg * 512
                if gj == 0:
                    # halves (512B elems, no small-desc penalty) so the
                    # first proj-mm starts sooner; wall tail lands between
                    nc.sync.dma_start(xtg[:, :, 0:256],
                                      xt[bj, :, :, base:base + 256])
                    nc.sync.dma_start(wall_sb[:, 4:10, :], wall[:, 4:10, :])
                    nc.sync.dma_start(wall_sb[:, 10:KC, :], wall[:, 10:KC, :])
                    nc.sync.dma_start(cos_sb[:], cosr[:, :, :])
                    nc.sync.dma_start(sin_sb[:], sinr[:, :, :])
                    nc.sync.dma_start(xtg[:, :, 256:512],
                                      xt[bj, :, :, base + 256:base + 512])
                else:
                    nc.sync.dma_start(xtg[:], xt[bj, :, :, base:base + 512])
                xtiles[(bj, tg)] = xtg
            if gi == 0:
                load_x(0)
                emit_tables()
            nxt = gi + 2
            if nxt < B * NP and nxt % 4 == 0:
                load_x(nxt)
            xtg = xtiles[(b, p // 4)]
            s = p % 4
            pq = pp_pq.tile([128, 384], F32, tag="pq", name=f"pq{b}_{p}")
            for kc in range(KC):
                nc.tensor.matmul(pq[:], xtg[:, kc, s * 128:(s + 1) * 128],
                                 wall_sb[:, kc, :],
                                 start=(kc == 0), stop=(kc == KC - 1))
            C[gi].update(b=b, p=p, pq=pq, bt=btiles[b])

        def emit_rope_muls(gi):
            pq, p = C[gi]["pq"], C[gi]["p"]
            qt, vsb, at = C[gi]["bt"]
            # one bulk copy releases the projection psum bank immediately;
            # rope and the V copy then read the cheaper SBUF copy
            qkf = qkvpool.tile([128, 384], F32, tag="qkf", name=f"qkf{gi}")
            nc.scalar.copy(qkf[:], pq[:])
            pear = qkf[:, 0:320].rearrange("p (h two i) -> p h two i", two=2, i=32)
            ev, od = pear[:, :, 0, :], pear[:, :, 1, :]
            cs = cos_sb[:, p, :].rearrange("p (h i) -> p h i", i=32)
            sn = sin_sb[:, p, :].rearrange("p (h i) -> p h i", i=32)
            ec = rpool.tile([128, 5, 32], F32, tag="ec")
            es = rpool.tile([128, 5, 32], F32, tag="es")
            oc = rpool.tile([128, 5, 32], F32, tag="oc")
            os_ = rpool.tile([128, 5, 32], F32, tag="os")
            nc.vector.tensor_mul(ec[:], ev, cs)
            nc.vector.tensor_mul(es[:], ev, sn)
            nc.vector.tensor_mul(oc[:], od, cs)
            nc.vector.tensor_mul(os_[:], od, sn)
            nc.vector.tensor_copy(vsb[:, p, 0:DH], qkf[:, 320:384])
            C[gi].update(ec=ec, es=es, oc=oc, os=os_)

        def emit_rope_comb(gi):
            c = C[gi]
            qkvb = qkvpool.tile([128, 5, 64], BF16, tag="qkvb", name=f"qkvb{gi}")
            qb = qkvb[:].rearrange("p h (two i) -> p h two i", two=2, i=32)
            nc.vector.tensor_sub(qb[:, :, 0, :], c["ec"][:], c["os"][:])
            nc.vector.tensor_add(qb[:, :, 1, :], c["es"][:], c["oc"][:])
            c["qkvb"] = qkvb

        def emit_tr_copies(gi):
            c = C[gi]
            qkvb, p = c["qkvb"], c["p"]
            qt, vsb, at = c["bt"]
            t = pp_sc.tile([128, 2, 512], F32, tag="sc", name=f"trq{gi}")
            tr = t[0:64, 0, 0:320].bitcast(BF16)  # [64, 640] view
            for h in range(5):
                nc.tensor.transpose(tr[:, h * 128:(h + 1) * 128],
                                    qkvb[:, h, :], identb[:])
            nc.vector.tensor_copy(
                qt[:, p, :, :],
                tr[:, 0:640].rearrange("p (h t) -> p h t", h=5))

        def emit_attention(gi):
            c = C[gi]
            p = c["p"]
            qt, vsb, at = c["bt"]
            pav = pp_av.tile([128, G, DH + 1], F32, tag="pav", name=f"pav{gi}")
            esbs = []
            for pr in range((p + 2) // 2):
                kcs = [k for k in (2 * pr, 2 * pr + 1) if k <= p]
                w = len(kcs)
                psc = pp_sc.tile([128, 2, 512], F32, tag="sc",
                                 name=f"sc{gi}_{pr}")
                for j, kc in enumerate(kcs):
                    nc.tensor.matmul(psc[:, j, :], qt[:, kc, 4, :],
                                     qt[:, p, 0:4, :], start=True, stop=True)
                esb = epool.tile([128, 2, 512], BF16, tag="esb",
                                 name=f"esb{gi}_{pr}")
                nc.scalar.activation(esb[:, 0:w, :], psc[:, 0:w, :],
                                     EXP, scale=0.125)
                if kcs[-1] == p:
                    nc.vector.tensor_mul(esb[:, w - 1, :], esb[:, w - 1, :],
                                         mask_sb[:])
                for j in range(w):
                    esbs.append(esb[:, j, :])
            # One OPEN psum accumulation group per bank at a time: run each
            # head's accumulation chain to completion before the next starts.
            for g in range(G):
                for kc in range(p + 1):
                    nc.tensor.matmul(pav[:, g, :],
                                     esbs[kc][:, g * 128:(g + 1) * 128],
                                     vsb[:, kc, :],
                                     start=(kc == 0), stop=(kc == p))
            # single bulk copy releases the pav bank immediately; the
            # normalize then reads the SBUF copy (cheaper access, no psum WAR)
            avs = spool.tile([128, G, DH + 1], F32, tag="avs", name=f"avs{gi}")
            nc.vector.tensor_copy(avs[:], pav[:])
            rcp = spool.tile([128, G], F32, tag="rcp", name=f"rcp{gi}")
            attn = apool.tile([128, G, DH], BF16, tag="attn", name=f"attn{gi}")
            nc.vector.reciprocal(rcp[:], avs[:, :, DH])
            for g in range(G):
                nc.vector.tensor_scalar_mul(attn[:, g, :], avs[:, g, 0:DH],
                                            rcp[:, g:g + 1])
            c["attn"] = attn

        def emit_at_tr(gi):
            c = C[gi]
            p, attn = c["p"], c["attn"]
            qt, vsb, at = c["bt"]
            tsl = slice(p * 128, (p + 1) * 128)
            # two heads per transpose: [128tok, 2*64] -> [128chan, 128tok],
            # landing directly in the o-proj's [chan, token] layout.
            t2 = pp_sc.tile([128, 2, 512], F32, tag="sc", name=f"tra{gi}")
            tr2 = t2[:, 0, 0:128].bitcast(BF16)  # [128, 256] bf16 view
            for cc in range(2):
                nc.tensor.transpose(tr2[:, cc * 128:(cc + 1) * 128],
                                    attn[:, 2 * cc:2 * cc + 2, :], identb[:])
            nc.vector.tensor_copy(
                at[:, :, tsl],
                tr2[:, 0:256].rearrange("p (c t) -> p c t", c=2))

        def emit_oproj(gi):
            c = C[gi]
            b, p = c["b"], c["p"]
            qt, vsb, at = c["bt"]
            tsl = slice(p * 128, (p + 1) * 128)
            osb = opool.tile([128, D], BF16, tag="osb", name=f"osb{gi}")
            for nt in range(4):
                nsl = slice(nt * 512, (nt + 1) * 512)
                po = pp_po.tile([128, 512], F32, tag="po")
                nc.tensor.matmul(po[:], at[:, 0, tsl], wot_sb[:, 0, nsl],
                                 start=True, stop=False)
                nc.tensor.matmul(po[:], at[:, 1, tsl], wot_sb[:, 1, nsl],
                                 start=False, stop=True)
                n_act = 3 if p < 6 else (2 if p < 10 else 1)
                if nt < n_act:
                    nc.scalar.copy(osb[:, nsl], po[:])
                else:
                    if gi == B * NP - 1 and nt % 2 == 0:
                    # final chunk: evacuate on ACT+DVE in parallel to
                    # shorten the post-compute drain chain
                    nc.scalar.copy(osb[:, nsl], po[:])
                else:
                    nc.vector.tensor_copy(osb[:, nsl], po[:])
                if nt % 2 == 1:  # write each half as soon as it's staged
                    nc.gpsimd.dma_start(
                        o[b, tsl, (nt - 1) * 512:(nt + 1) * 512],
                        osb[:, (nt - 1) * 512:(nt + 1) * 512])

        NG = B * NP
        for i in range(NG + 1):
            if i < NG:
                emit_proj(i)
            if i - 3 >= 0:
                emit_at_tr(i - 3)
            if 0 <= i - 1 < NG:
                emit_tr_copies(i - 1)
            if i < NG:
                emit_rope_muls(i)
                emit_rope_comb(i)
            if 0 <= i - 1 < NG:
                emit_attention(i - 1)
            if i - 4 >= 0:
                emit_oproj(i - 4)
            if i == NG:  # drain: flush remaining tails immediately
                for r in (NG - 2, NG - 1):
                    emit_at_tr(r)
                for r in (NG - 3, NG - 2, NG - 1):
                    emit_oproj(r)
    nc.compile()
    return nc


def _deinter(w):
    """[64, D] head rows, interleaved rope pairs -> [evens(32) | odds(32)]."""
    return np.concatenate([w[0::2], w[1::2]], axis=0)


def host_inputs(x, Wq, Wk, Wv, Wo):
    import ml_dtypes
    bf16 = ml_dtypes.bfloat16
    # [B, S, D] -> [B, 128, KC, S]: partition-major chunks of the d axis
    xtp = np.transpose(np.asarray(x, np.float32), (0, 2, 1))  # [B, D, S]
    xtp = np.ascontiguousarray(
        xtp.reshape(B, KC, 128, S).transpose(0, 2, 1, 3)).astype(bf16)

    inv = ROPE_BASE ** (-np.arange(0, DH, 2, dtype=np.float64) / DH)  # (32,)
    th = np.arange(S, dtype=np.float64)[:, None] * inv[None, :]       # (S, 32)
    cosr = np.tile(np.cos(th), (1, 5)).astype(np.float32)             # (S, 160)
    sinr = np.tile(np.sin(th), (1, 5)).astype(np.float32)
    cosr = np.ascontiguousarray(
        cosr.reshape(NP, 128, 160).transpose(1, 0, 2)).astype(bf16)   # (128,16,160)
    sinr = np.ascontiguousarray(
        sinr.reshape(NP, 128, 160).transpose(1, 0, 2)).astype(bf16)

    k_ = np.arange(128)[:, None]
    q_ = np.arange(128)[None, :]
    tril = (q_ >= k_).astype(np.float32)                               # (128,128)
    utri = np.ascontiguousarray(np.tile(tril, (1, 4))).astype(bf16)    # (128,512)
    identb = np.eye(128, dtype=np.float32).astype(bf16)

    in_maps = []
    for c in range(NCORES):
        rows = [_deinter(Wq[(4 * c + h) * DH:(4 * c + h + 1) * DH])
                for h in range(G)]
        rows.append(_deinter(Wk[c * DH:(c + 1) * DH]))
        rows.append(Wv[c * DH:(c + 1) * DH])
        wallc = np.concatenate(rows, axis=0).astype(np.float32)        # (384, D)
        wallc = np.ascontiguousarray(
            wallc.T.reshape(KC, 128, 384).transpose(1, 0, 2)).astype(bf16)
        # at[j, cc] holds head 2*cc + j//64, dh j%64 -> wot rows match
        wotc = np.empty((128, 2, D), np.float32)
        for cc in range(2):
            for half in range(2):
                head = 4 * c + 2 * cc + half
                wotc[half * 64:(half + 1) * 64, cc, :] = \
                    Wo[:, head * DH:(head + 1) * DH].T
        in_maps.append(dict(xt=xtp, wall=wallc, wot=wotc.astype(bf16),
                            cosr=cosr, sinr=sinr, utri=utri,
                            identb=identb))
    return in_maps


def kernel(**inputs):
    x = np.asarray(inputs["x"], dtype=np.float32)
    Wq = np.asarray(inputs["Wq"], dtype=np.float32)
    Wk = np.asarray(inputs["Wk"], dtype=np.float32)
    Wv = np.asarray(inputs["Wv"], dtype=np.float32)
    Wo = np.asarray(inputs["Wo"], dtype=np.float32)
    in_maps = host_inputs(x, Wq, Wk, Wv, Wo)
    if "nc" not in _cached:
        _cached["nc"] = build_nc()
    res = run_bass_kernel_spmd(_cached["nc"], in_maps, list(range(NCORES)))
    out = np.zeros((B, S, D), np.float32)
    for r in res.results:
        out += np.asarray(r["o"]).astype(np.float32)
    return out


# revision 47
# speedup vs baseline: 1.7784x; 1.0077x over previous
"""GQA (32 q heads / 8 kv heads, RoPE, causal) Trainium2 Bass kernel.

Sharding: tensor-parallel over kv heads — core c owns kv head c and q heads
4c..4c+3 for both batches. Each core computes a partial o-projection
(its 256 attn channels x all Wo columns) in bf16 and the host sums the 8
partials in f32.

Device-side structure (per core), 4-stage software pipeline over 128-token
chunks p (32 chunks across both batches) so the PE never waits on the
ACT/DVE rope/copy chains:
  iter i:  proj-mm(p=i) -> attn-transposes(i-3) -> QK-transposes(q=i-1)
           -> rope(p) -> scores/exp/AV+norm(q) -> o-proj(i-4)
  * QKV projection: bf16 x^T chunks (stationary) x bf16 fused W (moving 384)
    into f32 PSUM. RoPE reads the PSUM directly (DVE, f32, with a
    host-deinterleaved pair layout = contiguous even/odd halves) and writes
    a bf16 tile for the transposes — no separate PSUM evacuation.
  * Q/K transposed per head via PE (bf16, 1 c/row) into [dh, token] layout
    (k stored as head 4 of the same tensor); 4 q heads merged along the
    moving dim for scores.
  * Scores at 128x(4x128) causal granularity (no above-diagonal waste);
    key-chunk-paired exp on ACT (scale=1/8, no max subtraction needed, one
    [128,1024] op per two chunks amortizes the ACT access latency); the
    diagonal tile masked by one bf16 DVE multiply.
  * AV in [query, dh] layout: out [128q, 65] per head with a ones column in
    V giving the softmax denominator; full 128-partition output = 2x fewer
    PE cycles than the [dh, query] layout. Each head's PSUM accumulation
    chain runs to completion before the next starts (a PSUM bank supports
    only one open accumulation group). PSUM banks (pq/scores/pav/po) are
    released by single bulk copies so the next producer never waits on the
    consumer chains; normalization = DVE reciprocal + per-partition
    tensor_scalar multiplies off the SBUF copy (no PE broadcast).
  * attn transposed back to [chan, token] (2 heads per transpose) for the
    o-proj; o written as bf16 via Pool-issued (SWDGE) DMAs to avoid the
    serialized HWDGE path.
"""

import numpy as np
from contextlib import ExitStack

import concourse.bass as bass
from concourse import bacc
import concourse.mybir as mybir
import concourse.tile as tile
from concourse.bass_utils import run_bass_kernel_spmd

B, S, D = 2, 2048, 2048
DH = 64            # head dim
G = 4              # q heads per core (= per kv head)
NCORES = 8
NP = S // 128      # 16 token chunks of 128 per batch
KC = D // 128      # 16 contraction chunks
F32 = mybir.dt.float32
F32R = mybir.dt.float32r
BF16 = mybir.dt.bfloat16
ROPE_BASE = 10000.0

_cached = {}


def build_nc():
    nc = bacc.Bacc("TRN2", target_bir_lowering=False, debug=False)
    xt = nc.declare_dram_parameter("xt", [B, 128, KC, S], BF16, isOutput=False)
    wall = nc.declare_dram_parameter("wall", [128, KC, 384], BF16, isOutput=False)
    wot = nc.declare_dram_parameter("wot", [128, 2, D], BF16, isOutput=False)
    cosr = nc.declare_dram_parameter("cosr", [128, NP, 160], BF16, isOutput=False)
    sinr = nc.declare_dram_parameter("sinr", [128, NP, 160], BF16, isOutput=False)
    utri_d = nc.declare_dram_parameter("utri", [128, 512], BF16, isOutput=False)
    identb_d = nc.declare_dram_parameter("identb", [128, 128], BF16, isOutput=False)
    o = nc.declare_dram_parameter("o", [B, S, D], BF16, isOutput=True)

    EXP = mybir.ActivationFunctionType.Exp

    with tile.TileContext(nc) as tc, ExitStack() as ctx:
        wpool = ctx.enter_context(tc.tile_pool(name="weights", bufs=1))
        xpool = ctx.enter_context(tc.tile_pool(name="x", bufs=2))
        qkvpool = ctx.enter_context(tc.tile_pool(name="qkvb", bufs=3))
        rpool = ctx.enter_context(tc.tile_pool(name="rope", bufs=2))
        epool = ctx.enter_context(tc.tile_pool(name="exp", bufs=12))
        bpool = ctx.enter_context(tc.tile_pool(name="perb", bufs=2))
        apool = ctx.enter_context(tc.tile_pool(name="attn", bufs=4))
        spool = ctx.enter_context(tc.tile_pool(name="small", bufs=4))
        opool = ctx.enter_context(tc.tile_pool(name="osb", bufs=2))
        pp_pq = ctx.enter_context(tc.tile_pool(name="ppq", bufs=1, space="PSUM"))
        pp_sc = ctx.enter_context(tc.tile_pool(name="psc", bufs=2, space="PSUM"))
        pp_av = ctx.enter_context(tc.tile_pool(name="pav", bufs=1, space="PSUM"))
        pp_po = ctx.enter_context(tc.tile_pool(name="ppo", bufs=2, space="PSUM"))

        # ---- persistent weights/tables ----
        # wall first (needed by proj(0)); the rest are emitted after the
        # first x-tile DMA so startup isn't serialized behind weight loads
        # the first iterations don't need yet.
        wall_sb = wpool.tile([128, KC, 384], BF16, tag="wall")
        wot_sb = wpool.tile([128, 2, D], BF16, tag="wot")
        cos_sb = wpool.tile([128, NP, 160], BF16, tag="cos")
        sin_sb = wpool.tile([128, NP, 160], BF16, tag="sin")
        mask_sb = wpool.tile([128, 512], BF16, tag="mask")
        identb = wpool.tile([128, 128], BF16, tag="identb")
        nc.sync.dma_start(wall_sb[:, 0:4, :], wall[:, 0:4, :])

        def emit_tables():
            nc.sync.dma_start(identb[:], identb_d[:, :])
            nc.sync.dma_start(mask_sb[:], utri_d[:, :])
            nc.sync.dma_start(wot_sb[:], wot[:, :, :])

        # per-chunk state, filled by emit stages
        C = [dict() for _ in range(B * NP)]
        xtiles = {}
        btiles = {}

        def emit_proj(gi):
            b, p = gi // NP, gi % NP
            if p == 0:
                # qt holds the 4 roped q heads AND k (slot 4) in [dh, token]
                qt = bpool.tile([64, NP, 5, 128], BF16, tag="qt", name=f"qt{b}")
                vsb = bpool.tile([128, NP, DH + 1], BF16, tag="vsb", name=f"vsb{b}")
                at = bpool.tile([128, 2, S], BF16, tag="at", name=f"at{b}")
                nc.gpsimd.memset(vsb[:], 1.0)
                btiles[b] = (qt, vsb, at)
            def load_x(gj):
                bj, tg = gj // NP, (gj % NP) // 4
                xtg = xpool.tile([128, KC, 512], BF16, tag="x",
                                 name=f"x{bj}_{tg}")
                base = tg * 512
                if gj == 0:
                    # staged quarters (512B elems, no small-desc penalty):
                    # the first proj-mm only needs kc 0:8 of tokens 0:256,
                    # so it can start after ~2.6us instead of ~6.9us
                    nc.sync.dma_start(xtg[:, 0:8, 0:256],
                                      xt[bj, :, 0:8, base:base + 256])
                    nc.sync.dma_start(xtg[:, 8:KC, 0:256],
                                      xt[bj, :, 8:KC, base:base + 256])
                    nc.sync.dma_start(wall_sb[:, 4:10, :], wall[:, 4:10, :])
                    nc.sync.dma_start(wall_sb[:, 10:KC, :], wall[:, 10:KC, :])
                    nc.sync.dma_start(cos_sb[:], cosr[:, :, :])
                    nc.sync.dma_start(sin_sb[:], sinr[:, :, :])
                    nc.sync.dma_start(xtg[:, :, 256:512],
                                      xt[bj, :, :, base + 256:base + 512])
                else:
                    nc.sync.dma_start(xtg[:], xt[bj, :, :, base:base + 512])
                xtiles[(bj, tg)] = xtg
            if gi == 0:
                load_x(0)
                emit_tables()
            nxt = gi + 3
            if nxt < B * NP and nxt % 4 == 0:
                load_x(nxt)
            xtg = xtiles[(b, p // 4)]
            s = p % 4
            pq = pp_pq.tile([128, 384], F32, tag="pq", name=f"pq{b}_{p}")
            for kc in range(KC):
                nc.tensor.matmul(pq[:], xtg[:, kc, s * 128:(s + 1) * 128],
                                 wall_sb[:, kc, :],
                                 start=(kc == 0), stop=(kc == KC - 1))
            C[gi].update(b=b, p=p, pq=pq, bt=btiles[b])

        def emit_rope_muls(gi):
            pq, p = C[gi]["pq"], C[gi]["p"]
            qt, vsb, at = C[gi]["bt"]
            # one bulk copy releases the projection psum bank immediately;
            # rope and the V copy then read the cheaper SBUF copy
            qkf = qkvpool.tile([128, 384], F32, tag="qkf", name=f"qkf{gi}")
            nc.scalar.copy(qkf[:], pq[:])
            pear = qkf[:, 0:320].rearrange("p (h two i) -> p h two i", two=2, i=32)
            ev, od = pear[:, :, 0, :], pear[:, :, 1, :]
            cs = cos_sb[:, p, :].rearrange("p (h i) -> p h i", i=32)
            sn = sin_sb[:, p, :].rearrange("p (h i) -> p h i", i=32)
            ec = rpool.tile([128, 5, 32], F32, tag="ec")
            es = rpool.tile([128, 5, 32], F32, tag="es")
            oc = rpool.tile([128, 5, 32], F32, tag="oc")
            os_ = rpool.tile([128, 5, 32], F32, tag="os")
            nc.vector.tensor_mul(ec[:], ev, cs)
            nc.vector.tensor_mul(es[:], ev, sn)
            nc.vector.tensor_mul(oc[:], od, cs)
            nc.vector.tensor_mul(os_[:], od, sn)
            nc.vector.tensor_copy(vsb[:, p, 0:DH], qkf[:, 320:384])
            C[gi].update(ec=ec, es=es, oc=oc, os=os_)

        def emit_rope_comb(gi):
            c = C[gi]
            qkvb = qkvpool.tile([128, 5, 64], BF16, tag="qkvb", name=f"qkvb{gi}")
            qb = qkvb[:].rearrange("p h (two i) -> p h two i", two=2, i=32)
            nc.vector.tensor_sub(qb[:, :, 0, :], c["ec"][:], c["os"][:])
            nc.vector.tensor_add(qb[:, :, 1, :], c["es"][:], c["oc"][:])
            c["qkvb"] = qkvb

        def emit_tr_copies(gi):
            c = C[gi]
            qkvb, p = c["qkvb"], c["p"]
            qt, vsb, at = c["bt"]
            t = pp_sc.tile([128, 2, 512], F32, tag="sc", name=f"trq{gi}")
            tr = t[0:64, 0, 0:320].bitcast(BF16)  # [64, 640] view
            for h in range(5):
                nc.tensor.transpose(tr[:, h * 128:(h + 1) * 128],
                                    qkvb[:, h, :], identb[:])
            nc.vector.tensor_copy(
                qt[:, p, :, :],
                tr[:, 0:640].rearrange("p (h t) -> p h t", h=5))

        def emit_attention(gi):
            c = C[gi]
            p = c["p"]
            qt, vsb, at = c["bt"]
            pav = pp_av.tile([128, G, DH + 1], F32, tag="pav", name=f"pav{gi}")
            esbs = []
            for pr in range((p + 2) // 2):
                kcs = [k for k in (2 * pr, 2 * pr + 1) if k <= p]
                w = len(kcs)
                psc = pp_sc.tile([128, 2, 512], F32, tag="sc",
                                 name=f"sc{gi}_{pr}")
                for j, kc in enumerate(kcs):
                    nc.tensor.matmul(psc[:, j, :], qt[:, kc, 4, :],
                                     qt[:, p, 0:4, :], start=True, stop=True)
                esb = epool.tile([128, 2, 512], BF16, tag="esb",
                                 name=f"esb{gi}_{pr}")
                nc.scalar.activation(esb[:, 0:w, :], psc[:, 0:w, :],
                                     EXP, scale=0.125)
                if kcs[-1] == p:
                    nc.vector.tensor_mul(esb[:, w - 1, :], esb[:, w - 1, :],
                                         mask_sb[:])
                for j in range(w):
                    esbs.append(esb[:, j, :])
            # One OPEN psum accumulation group per bank at a time: run each
            # head's accumulation chain to completion before the next starts.
            for g in range(G):
                for kc in range(p + 1):
                    nc.tensor.matmul(pav[:, g, :],
                                     esbs[kc][:, g * 128:(g + 1) * 128],
                                     vsb[:, kc, :],
                                     start=(kc == 0), stop=(kc == p))
            # single bulk copy releases the pav bank immediately; the
            # normalize then reads the SBUF copy (cheaper access, no psum WAR)
            avs = spool.tile([128, G, DH + 1], F32, tag="avs", name=f"avs{gi}")
            nc.vector.tensor_copy(avs[:], pav[:])
            rcp = spool.tile([128, G], F32, tag="rcp", name=f"rcp{gi}")
            attn = apool.tile([128, G, DH], BF16, tag="attn", name=f"attn{gi}")
            nc.vector.reciprocal(rcp[:], avs[:, :, DH])
            for g in range(G):
                nc.vector.tensor_scalar_mul(attn[:, g, :], avs[:, g, 0:DH],
                                            rcp[:, g:g + 1])
            c["attn"] = attn

        def emit_at_tr(gi):
            c = C[gi]
            p, attn = c["p"], c["attn"]
            qt, vsb, at = c["bt"]
            tsl = slice(p * 128, (p + 1) * 128)
            # two heads per transpose: [128tok, 2*64] -> [128chan, 128tok],
            # landing directly in the o-proj's [chan, token] layout.
            t2 = pp_sc.tile([128, 2, 512], F32, tag="sc", name=f"tra{gi}")
            tr2 = t2[:, 0, 0:128].bitcast(BF16)  # [128, 256] bf16 view
            for cc in range(2):
                nc.tensor.transpose(tr2[:, cc * 128:(cc + 1) * 128],
                                    attn[:, 2 * cc:2 * cc + 2, :], identb[:])
            nc.vector.tensor_copy(
                at[:, :, tsl],
                tr2[:, 0:256].rearrange("p (c t) -> p c t", c=2))

        def emit_oproj(gi):
            c = C[gi]
            b, p = c["b"], c["p"]
            qt, vsb, at = c["bt"]
            tsl = slice(p * 128, (p + 1) * 128)
            osb = opool.tile([128, D], BF16, tag="osb", name=f"osb{gi}")
            for nt in range(4):
                nsl = slice(nt * 512, (nt + 1) * 512)
                po = pp_po.tile([128, 512], F32, tag="po")
                nc.tensor.matmul(po[:], at[:, 0, tsl], wot_sb[:, 0, nsl],
                                 start=True, stop=False)
                nc.tensor.matmul(po[:], at[:, 1, tsl], wot_sb[:, 1, nsl],
                                 start=False, stop=True)
                n_act = 3 if p < 6 else (2 if p < 10 else 1)
                if nt < n_act:
                    nc.scalar.copy(osb[:, nsl], po[:])
                else:
                    last = gi >= B * NP - 2
                if last and nt % 2 == 0:
                    # final chunk: evacuate on ACT+DVE in parallel to
                    # shorten the post-compute drain chain
                    nc.scalar.copy(osb[:, nsl], po[:])
                else:
                    nc.vector.tensor_copy(osb[:, nsl], po[:])
                if last:
                    # per-quarter writes on the (now idle) SP queue: the
                    # final DMA chain starts right after each evacuation
                    nc.sync.dma_start(o[b, tsl, nsl], osb[:, nsl])
                elif nt % 2 == 1:  # write each half as soon as it's staged
                    nc.gpsimd.dma_start(
                        o[b, tsl, (nt - 1) * 512:(nt + 1) * 512],
                        osb[:, (nt - 1) * 512:(nt + 1) * 512])

        NG = B * NP
        for i in range(NG + 1):
            if i < NG:
                emit_proj(i)
            if i - 3 >= 0:
                emit_at_tr(i - 3)
            if 0 <= i - 1 < NG:
                emit_tr_copies(i - 1)
            if i < NG:
                emit_rope_muls(i)
                emit_rope_comb(i)
            if 0 <= i - 1 < NG:
                emit_attention(i - 1)
            if i - 4 >= 0:
                emit_oproj(i - 4)
            if i == NG:  # drain: flush remaining tails immediately
                for r in (NG - 2, NG - 1):
                    emit_at_tr(r)
                for r in (NG - 3, NG - 2, NG - 1):
                    emit_oproj(r)
    nc.compile()
    return nc


def _deinter(w):
    """[64, D] head rows, interleaved rope pairs -> [evens(32) | odds(32)]."""
    return np.concatenate([w[0::2], w[1::2]], axis=0)


def host_inputs(x, Wq, Wk, Wv, Wo):
    import ml_dtypes
    bf16 = ml_dtypes.bfloat16
    # [B, S, D] -> [B, 128, KC, S]: partition-major chunks of the d axis
    xtp = np.transpose(np.asarray(x, np.float32), (0, 2, 1))  # [B, D, S]
    xtp = np.ascontiguousarray(
        xtp.reshape(B, KC, 128, S).transpose(0, 2, 1, 3)).astype(bf16)

    inv = ROPE_BASE ** (-np.arange(0, DH, 2, dtype=np.float64) / DH)  # (32,)
    th = np.arange(S, dtype=np.float64)[:, None] * inv[None, :]       # (S, 32)
    cosr = np.tile(np.cos(th), (1, 5)).astype(np.float32)             # (S, 160)
    sinr = np.tile(np.sin(th), (1, 5)).astype(np.float32)
    cosr = np.ascontiguousarray(
        cosr.reshape(NP, 128, 160).transpose(1, 0, 2)).astype(bf16)   # (128,16,160)
    sinr = np.ascontiguousarray(
        sinr.reshape(NP, 128, 160).transpose(1, 0, 2)).astype(bf16)

    k_ = np.arange(128)[:, None]
    q_ = np.arange(128)[None, :]
    tril = (q_ >= k_).astype(np.float32)                               # (128,128)
    utri = np.ascontiguousarray(np.tile(tril, (1, 4))).astype(bf16)    # (128,512)
    identb = np.eye(128, dtype=np.float32).astype(bf16)

    in_maps = []
    for c in range(NCORES):
        rows = [_deinter(Wq[(4 * c + h) * DH:(4 * c + h + 1) * DH])
                for h in range(G)]
        rows.append(_deinter(Wk[c * DH:(c + 1) * DH]))
        rows.append(Wv[c * DH:(c + 1) * DH])
        wallc = np.concatenate(rows, axis=0).astype(np.float32)        # (384, D)
        wallc = np.ascontiguousarray(
            wallc.T.reshape(KC, 128, 384).transpose(1, 0, 2)).astype(bf16)
        # at[j, cc] holds head 2*cc + j//64, dh j%64 -> wot rows match
        wotc = np.empty((128, 2, D), np.float32)
        for cc in range(2):
            for half in range(2):
                head = 4 * c + 2 * cc + half
                wotc[half * 64:(half + 1) * 64, cc, :] = \
                    Wo[:, head * DH:(head + 1) * DH].T
        in_maps.append(dict(xt=xtp, wall=wallc, wot=wotc.astype(bf16),
                            cosr=cosr, sinr=sinr, utri=utri,
                            identb=identb))
    return in_maps


def kernel(**inputs):
    x = np.asarray(inputs["x"], dtype=np.float32)
    Wq = np.asarray(inputs["Wq"], dtype=np.float32)
    Wk = np.asarray(inputs["Wk"], dtype=np.float32)
    Wv = np.asarray(inputs["Wv"], dtype=np.float32)
    Wo = np.asarray(inputs["Wo"], dtype=np.float32)
    in_maps = host_inputs(x, Wq, Wk, Wv, Wo)
    if "nc" not in _cached:
        _cached["nc"] = build_nc()
    res = run_bass_kernel_spmd(_cached["nc"], in_maps, list(range(NCORES)))
    out = np.zeros((B, S, D), np.float32)
    for r in res.results:
        out += np.asarray(r["o"]).astype(np.float32)
    return out
